# revision 11
# baseline (speedup 1.0000x reference)
"""Trainium2 Bass kernel for causal multi-head attention (B=2,S=2048,D=1024,H=16).

Sharding: batch*head-group across 8 cores. Core c = (b, g) = (c//4, c%4) computes
heads [4g, 4g+4) of batch b.

Device computes, per core (fp16 matmuls, fp32 PSUM):
  QT/KT = (X @ W)^T via W-stationary matmuls    -> [256, 2048] fp16 in SBUF
  V     = X @ Wv via XT-stationary matmuls      -> [2048, 4, 68] fp16 (+ones col)
  scoresT[k, q] = K_h @ Q_h^T (K=64 contraction), exp on ScalarE (scale=1/8,
  bias=-4), partial causal blocks masked by multiplying 0/1 patterns,
  PV matmul with ones-column -> x'^T[65, q] where row 64 = softmax denominator l,
  r = 1/l, broadcast via ones-outer-product matmul, x^T normalized,
  out_partial = x^T.T @ Wo_rows.
Host: gathers fp16 probs (unnormalized, [k,q] layout), multiplies by r,
transposes to [q,k]; sums the 4 partial outputs per batch and adds bias.
"""

import numpy as np
from contextlib import ExitStack

import concourse.bacc as bacc
import concourse.tile as tile
from concourse import mybir
from concourse.bass_utils import run_bass_kernel_spmd

B, S, D, H, HD = 2, 2048, 1024, 16, 64
NH = 4                # heads per core
G = H // NH           # head groups (cores per batch)
DPC = NH * HD         # 256 output dims per core
NKT, KTW = 16, 128    # key tiles
NQC, QCW = 4, 512     # query chunks
NPT = D // 128        # projection contraction tiles (8)
VPW = 68              # per-head V stride (64 data + 1 ones + 3 pad, 4B aligned)
F16 = mybir.dt.float16
F32 = mybir.dt.float32
F32R = mybir.dt.float32r
EXP_BIAS = -4.0
SCALE = 1.0 / 8.0     # 1/sqrt(HD)

AFT = mybir.ActivationFunctionType


def _block_structure(mask):
    """Classify (kt, qc) blocks from the actual mask values.

    Returns (act, par, plist):
      act[kt][qc]  - block has any attendable entry in any batch
      par[kt][qc]  - block needs masking (not all-ones in every batch)
      plist        - ordered list of (kt, qc) partial blocks
    """
    m = mask.reshape(B, NQC, QCW, NKT, KTW)
    any_b = m.any(axis=(2, 4))   # [B, NQC, NKT]
    all_b = m.all(axis=(2, 4))
    act = any_b.any(axis=0).T    # [NKT, NQC]
    full = all_b.all(axis=0).T
    par = act & ~full
    plist = [(kt, qc) for kt in range(NKT) for qc in range(NQC) if par[kt, qc]]
    return act, par, plist


def _build(act, par, plist, n_pat, loop_n=1):
    pidx = {kq: i for i, kq in enumerate(plist)}
    first_kt = {}
    last_kt = {}
    for qc in range(NQC):
        kts = [kt for kt in range(NKT) if act[kt, qc]]
        assert kts, f"query chunk {qc} has no attendable keys"
        first_kt[qc], last_kt[qc] = kts[0], kts[-1]

    nc = bacc.Bacc("TRN2", target_bir_lowering=False, debug=False, num_devices=8)
    qT = nc.dram_tensor("qT", [D, S], F16, kind="ExternalInput").ap()
    kT = nc.dram_tensor("kT", [D, S], F16, kind="ExternalInput").ap()
    vT = nc.dram_tensor("vT", [D, S], F16, kind="ExternalInput").ap()
    wq = nc.dram_tensor("wq", [D, DPC], F16, kind="ExternalInput").ap()
    wk = nc.dram_tensor("wk", [D, DPC], F16, kind="ExternalInput").ap()
    wv = nc.dram_tensor("wv", [D, DPC], F16, kind="ExternalInput").ap()
    wo = nc.dram_tensor("wo", [DPC, D], F16, kind="ExternalInput").ap()
    mpat = nc.dram_tensor("mpat", [n_pat, KTW, QCW], F16, kind="ExternalInput").ap()
    probs = nc.dram_tensor("probs", [NH, S, S], F16, kind="ExternalOutput").ap()
    outp = nc.dram_tensor("outp", [S, D], F32, kind="ExternalOutput").ap()
    rvec = nc.dram_tensor("rvec", [1, NH, S], F32, kind="ExternalOutput").ap()

    with tile.TileContext(nc) as tc, ExitStack() as ctx:
        singles = ctx.enter_context(tc.tile_pool(name="singles", bufs=1))

        # --- persistent tiles ---
        qTp = [singles.tile([128, S], F16, tag=f"qTp{t}", name=f"qTp{t}") for t in range(2)]
        kTp = [singles.tile([128, S], F16, tag=f"kTp{t}", name=f"kTp{t}") for t in range(2)]
        vp = singles.tile([128, NKT, NH, VPW], F16, tag="vp")
        xn = [singles.tile([128, S], F16, tag=f"xn{t}", name=f"xn{t}") for t in range(2)]
        mpat_sb = singles.tile([128, n_pat, QCW], F16, tag="mpat")
        r_sb = singles.tile([1, NH, S], F32, tag="r")
        ones_sb = singles.tile([1, HD], F16, tag="ones")
        r16_sb = singles.tile([1, NH, S], F16, tag="r16")
        ebias_sb = singles.tile([128, 1], F32, tag="ebias")
        wo_sb = [singles.tile([128, D], F16, tag=f"wo{t}", name=f"wo{t}") for t in range(2)]

        nc.vector.memset(ones_sb[:], 1.0)
        nc.vector.memset(ebias_sb[:], EXP_BIAS)
        nc.vector.memset(vp[:, :, :, HD:HD + 1], 1.0)
        nc.sync.dma_start(out=mpat_sb[:], in_=mpat.rearrange("n p f -> p n f"))
        for ct in range(2):
            nc.sync.dma_start(out=wo_sb[ct][:], in_=wo[ct * 128:(ct + 1) * 128, :])

        loop_ctx = tc.For_i(0, loop_n) if loop_n > 1 else None
        if loop_ctx is not None:
            ctx.enter_context(loop_ctx)

        # ============ phase 1: projections ============
        with tc.tile_pool(name="xin", bufs=10) as xin, \
             tc.tile_pool(name="wsb", bufs=3) as wsb, \
             tc.tile_pool(name="pp1", bufs=2, space="PSUM") as pp1:

            def load_w(src):
                w_t = wsb.tile([128, NPT, DPC], F16, tag="w")
                nc.sync.dma_start(out=w_t[:], in_=src.rearrange("(t p) n -> p t n", p=128))
                return w_t

            def load_x(src):
                tiles = []
                for t8 in range(NPT):
                    x_t = xin.tile([128, S], F16, tag="xin")
                    nc.sync.dma_start(out=x_t[:], in_=src[t8 * 128:(t8 + 1) * 128, :])
                    tiles.append(x_t)
                return tiles

            def proj_T(x_tiles, w_t, out_tiles):
                # out^T[n, q] = W^T @ X^T ; W stationary
                for nt in range(2):
                    for qc in range(NQC):
                        ps = pp1.tile([128, QCW], F32, tag="pp")
                        for t8 in range(NPT):
                            nc.tensor.matmul(
                                ps[:],
                                lhsT=w_t[:, t8, nt * 128:(nt + 1) * 128],
                                rhs=x_tiles[t8][:, qc * QCW:(qc + 1) * QCW],
                                start=(t8 == 0), stop=(t8 == NPT - 1))
                        nc.vector.tensor_copy(
                            out_tiles[nt][:, qc * QCW:(qc + 1) * QCW], ps[:])

            wq_t = load_w(wq)
            q_tiles = load_x(qT)
            proj_T(q_tiles, wq_t, qTp)

            wk_t = load_w(wk)
            k_tiles = load_x(kT)
            proj_T(k_tiles, wk_t, kTp)

            wv_t = load_w(wv)
            v_tiles = load_x(vT)
            # V[m, n] = X @ Wv ; X^T stationary
            for mt in range(NKT):
                ps = pp1.tile([128, DPC], F32, tag="pp")
                for t8 in range(NPT):
                    nc.tensor.matmul(
                        ps[:],
                        lhsT=v_tiles[t8][:, mt * 128:(mt + 1) * 128],
                        rhs=wv_t[:, t8, :],
                        start=(t8 == 0), stop=(t8 == NPT - 1))
                nc.vector.tensor_copy(
                    vp[:, mt, :, 0:HD],
                    ps[:].rearrange("p (h d) -> p h d", h=NH))

        # ============ phase 2: attention ============
        with tc.tile_pool(name="row", bufs=3) as rowp, \
             tc.tile_pool(name="rbs", bufs=2) as rbp, \
             tc.tile_pool(name="ps2", bufs=2, space="PSUM") as scp, \
             tc.tile_pool(name="xpp", bufs=4, space="PSUM") as xpp:

            for h in range(NH):
                t, r0 = h // 2, 64 * (h % 2)
                xp_tiles = {qc: xpp.tile([65, QCW], F32, tag="xp", name=f"xp{qc}") for qc in range(NQC)}
                for kt in range(NKT):
                    aqcs = [qc for qc in range(NQC) if act[kt, qc]]
                    if not aqcs:
                        continue
                    row = rowp.tile([128, S], F16, tag="row")
                    for qc in aqcs:
                        qs = slice(qc * QCW, (qc + 1) * QCW)
                        ps = scp.tile([128, QCW], F32, tag="sc")
                        nc.tensor.matmul(
                            ps[:],
                            lhsT=kTp[t][r0:r0 + 64, kt * KTW:(kt + 1) * KTW],
                            rhs=qTp[t][r0:r0 + 64, qs],
                            start=True, stop=True)
                        nc.scalar.activation(
                            out=row[:, qs], in_=ps[:], func=AFT.Exp,
                            bias=ebias_sb[:], scale=SCALE)
                        if par[kt, qc]:
                            nc.vector.tensor_mul(
                                row[:, qs], row[:, qs],
                                mpat_sb[:, pidx[(kt, qc)], :])
                        nc.tensor.matmul(
                            xp_tiles[qc][0:65, :],
                            lhsT=vp[:, kt, h, 0:HD + 1],
                            rhs=row[:, qs],
                            start=(kt == first_kt[qc]), stop=(kt == last_kt[qc]))
                    # store contiguous runs of active chunks
                    run0 = aqcs[0]
                    prev = aqcs[0]
                    runs = []
                    for qc in aqcs[1:]:
                        if qc == prev + 1:
                            prev = qc
                        else:
                            runs.append((run0, prev))
                            run0 = prev = qc
                    runs.append((run0, prev))
                    for (a, z) in runs:
                        nc.sync.dma_start(
                            out=probs[h, kt * KTW:(kt + 1) * KTW, a * QCW:(z + 1) * QCW],
                            in_=row[:, a * QCW:(z + 1) * QCW])
                for qc in range(NQC):
                    qs = slice(qc * QCW, (qc + 1) * QCW)
                    xp = xp_tiles[qc]
                    nc.vector.reciprocal(r_sb[0:1, h, qs], xp[64:65, :])
                    with nc.allow_low_precision(reason="fp16 r for broadcast"):
                        nc.vector.reciprocal(r16_sb[0:1, h, qs], xp[64:65, :])
                    rb = scp.tile([64, QCW], F32, tag="sc")
                    nc.tensor.matmul(
                        rb[0:64, :],
                        lhsT=ones_sb[0:1, :],
                        rhs=r16_sb[0:1, h, qs],
                        start=True, stop=True)
                    rbs = rbp.tile([64, QCW], F32, tag="rbs")
                    nc.vector.tensor_copy(rbs[:], rb[0:64, :])
                    nc.vector.tensor_tensor(
                        out=xn[t][r0:r0 + 64, qs], in0=xp[0:64, :], in1=rbs[:],
                        op=mybir.AluOpType.mult)
            nc.sync.dma_start(out=rvec[:, :, :], in_=r_sb[:])

        # ============ phase 3: output projection ============
        with tc.tile_pool(name="osb", bufs=3) as osbp, \
             tc.tile_pool(name="pp3", bufs=4, space="PSUM") as pp3:
            for mt in range(NKT):
                osb = osbp.tile([128, D], F32, tag="osb")
                for dmc in range(2):
                    ps = pp3.tile([128, 512], F32, tag="op")
                    for ct in range(2):
                        nc.tensor.matmul(
                            ps[:],
                            lhsT=xn[ct][:, mt * 128:(mt + 1) * 128],
                            rhs=wo_sb[ct][:, dmc * 512:(dmc + 1) * 512],
                            start=(ct == 0), stop=(ct == 1))
                    nc.scalar.copy(osb[:, dmc * 512:(dmc + 1) * 512], ps[:])
                nc.sync.dma_start(out=outp[mt * 128:(mt + 1) * 128, :], in_=osb[:])

    nc.compile()
    return nc


_CACHE = {}


def _get_program(act, par, plist, n_pat, loop_n=1):
    key = (act.tobytes(), par.tobytes(), n_pat, loop_n)
    if key not in _CACHE:
        _CACHE[key] = _build(act, par, plist, n_pat, loop_n)
    return _CACHE[key]


def kernel(query, key, value, mask, Wq, Wk, Wv, Wo, bo, _want_trace=False,
           _loop_n=1):
    query = np.asarray(query, np.float32)
    key = np.asarray(key, np.float32)
    value = np.asarray(value, np.float32)
    mask = np.asarray(mask)
    Wq = np.asarray(Wq, np.float32)
    Wk = np.asarray(Wk, np.float32)
    Wv = np.asarray(Wv, np.float32)
    Wo = np.asarray(Wo, np.float32)
    bo = np.asarray(bo, np.float32)

    act, par, plist = _block_structure(mask != 0)
    n_pat = max(len(plist), 1)
    nc = _get_program(act, par, plist, n_pat, _loop_n)

    in_maps = []
    for b in range(B):
        qTb = query[b].T.astype(np.float16)
        kTb = key[b].T.astype(np.float16)
        vTb = value[b].T.astype(np.float16)
        if plist:
            mp = np.stack([
                mask[b, qc * QCW:(qc + 1) * QCW, kt * KTW:(kt + 1) * KTW].T
                for (kt, qc) in plist]).astype(np.float16)
        else:
            mp = np.ones((1, KTW, QCW), np.float16)
        for g in range(G):
            in_maps.append({
                "qT": qTb, "kT": kTb, "vT": vTb,
                "wq": Wq[:, g * DPC:(g + 1) * DPC].astype(np.float16),
                "wk": Wk[:, g * DPC:(g + 1) * DPC].astype(np.float16),
                "wv": Wv[:, g * DPC:(g + 1) * DPC].astype(np.float16),
                "wo": Wo[g * DPC:(g + 1) * DPC, :].astype(np.float16),
                "mpat": mp,
            })

    kwargs = {}
    if _want_trace:
        kwargs = dict(trace=True, trace_cores=[0])
    import time as _time
    _t0 = _time.time()
    res = run_bass_kernel_spmd(nc, in_maps, core_ids=list(range(8)), **kwargs)
    kernel._last_exec_wall = _time.time() - _t0

    out = np.zeros((B, S, D), np.float32)
    attn = np.zeros((B, H, S, S), np.float32)
    for c in range(8):
        b, g = divmod(c, G)
        rr = res.results[c]
        out[b] += rr["outp"]
        rv = rr["rvec"].reshape(NH, S)
        pr = rr["probs"]
        for lh in range(NH):
            h = g * NH + lh
            a = pr[lh].astype(np.float32)
            a *= rv[lh][None, :]
            attn[b, h] = a.T
    out += bo[None, None, :]
    if _want_trace:
        kernel._last_results = res
    return out, attn


# revision 18
# speedup vs baseline: 14.0863x; 14.0863x over previous
"""Trainium2 Bass kernel for causal multi-head attention (B=2,S=2048,D=1024,H=16).

Sharding: batch*head-group across 8 cores. Core c = (b, g) = (c//4, c%4) computes
heads [4g, 4g+4) of batch b.

Device computes, per core (fp16 matmuls, fp32 PSUM):
  QT/KT = (X @ W)^T via W-stationary matmuls    -> [256, 2048] fp16 in SBUF
  V     = X @ Wv via XT-stationary matmuls      -> [2048, 4, 68] fp16 (+ones col)
  scoresT[k, q] = K_h @ Q_h^T (K=64 contraction), exp on ScalarE (scale=1/8,
  bias=-4), partial causal blocks masked by multiplying 0/1 patterns,
  PV matmul with ones-column -> x'^T[65, q] where row 64 = softmax denominator l,
  r = 1/l, broadcast via ones-outer-product matmul, x^T normalized,
  out_partial = x^T.T @ Wo_rows.
Host: gathers fp16 probs (unnormalized, [k,q] layout), multiplies by r,
transposes to [q,k]; sums the 4 partial outputs per batch and adds bias.
"""

import numpy as np
from contextlib import ExitStack

import concourse.bacc as bacc
import concourse.tile as tile
from concourse import mybir
from concourse.bass_utils import run_bass_kernel_spmd

B, S, D, H, HD = 2, 2048, 1024, 16, 64
NH = 4                # heads per core
G = H // NH           # head groups (cores per batch)
DPC = NH * HD         # 256 output dims per core
NKT, KTW = 16, 128    # key tiles
NQC, QCW = 4, 512     # query chunks
NPT = D // 128        # projection contraction tiles (8)
VPW = 68              # per-head V stride (64 data + 1 ones + 3 pad, 4B aligned)
F16 = mybir.dt.float16
F32 = mybir.dt.float32
F32R = mybir.dt.float32r
EXP_BIAS = -4.0
SCALE = 1.0 / 8.0     # 1/sqrt(HD)

AFT = mybir.ActivationFunctionType


def _block_structure(mask):
    """Classify (kt, qc) blocks from the actual mask values.

    Returns (act, par, plist):
      act[kt][qc]  - block has any attendable entry in any batch
      par[kt][qc]  - block needs masking (not all-ones in every batch)
      plist        - ordered list of (kt, qc) partial blocks
    """
    m = mask.reshape(B, NQC, QCW, NKT, KTW)
    any_b = m.any(axis=(2, 4))   # [B, NQC, NKT]
    all_b = m.all(axis=(2, 4))
    act = any_b.any(axis=0).T    # [NKT, NQC]
    full = all_b.all(axis=0).T
    par = act & ~full
    plist = [(kt, qc) for kt in range(NKT) for qc in range(NQC) if par[kt, qc]]
    return act, par, plist


def _build(act, par, plist, n_pat, loop_n=1, loop_phases=(1, 2, 3)):
    pidx = {kq: i for i, kq in enumerate(plist)}
    first_kt = {}
    last_kt = {}
    for qc in range(NQC):
        kts = [kt for kt in range(NKT) if act[kt, qc]]
        assert kts, f"query chunk {qc} has no attendable keys"
        first_kt[qc], last_kt[qc] = kts[0], kts[-1]

    nc = bacc.Bacc("TRN2", target_bir_lowering=False, debug=False, num_devices=8)
    qT = nc.dram_tensor("qT", [D, S], F16, kind="ExternalInput").ap()
    kT = nc.dram_tensor("kT", [D, S], F16, kind="ExternalInput").ap()
    vT = nc.dram_tensor("vT", [D, S], F16, kind="ExternalInput").ap()
    wq = nc.dram_tensor("wq", [D, DPC], F16, kind="ExternalInput").ap()
    wk = nc.dram_tensor("wk", [D, DPC], F16, kind="ExternalInput").ap()
    wv = nc.dram_tensor("wv", [D, DPC], F16, kind="ExternalInput").ap()
    wo = nc.dram_tensor("wo", [DPC, D], F16, kind="ExternalInput").ap()
    mpat = nc.dram_tensor("mpat", [n_pat, KTW, QCW], F16, kind="ExternalInput").ap()
    probs = nc.dram_tensor("probs", [NH, S, S], F16, kind="ExternalOutput").ap()
    outp = nc.dram_tensor("outp", [S, D], F32, kind="ExternalOutput").ap()
    rvec = nc.dram_tensor("rvec", [1, NH, S], F32, kind="ExternalOutput").ap()

    with tile.TileContext(nc) as tc, ExitStack() as ctx:
        singles = ctx.enter_context(tc.tile_pool(name="singles", bufs=1))

        # --- persistent tiles ---
        qTp = [singles.tile([128, S], F16, tag=f"qTp{t}", name=f"qTp{t}") for t in range(2)]
        kTp = [singles.tile([128, S], F16, tag=f"kTp{t}", name=f"kTp{t}") for t in range(2)]
        vp = singles.tile([128, NKT, NH, VPW], F16, tag="vp")
        xn = [singles.tile([128, S], F16, tag=f"xn{t}", name=f"xn{t}") for t in range(2)]
        mpat_sb = singles.tile([128, n_pat, QCW], F16, tag="mpat")
        r_sb = singles.tile([1, NH, S], F32, tag="r")
        ones_sb = singles.tile([1, HD], F16, tag="ones")
        r16_sb = singles.tile([1, NH, S], F16, tag="r16")
        ebias_sb = singles.tile([128, 1], F32, tag="ebias")
        wo_sb = [singles.tile([128, D], F16, tag=f"wo{t}", name=f"wo{t}") for t in range(2)]

        nc.vector.memset(ones_sb[:], 1.0)
        nc.vector.memset(ebias_sb[:], EXP_BIAS)
        nc.vector.memset(vp[:, :, :, HD:HD + 1], 1.0)
        nc.sync.dma_start(out=mpat_sb[:], in_=mpat.rearrange("n p f -> p n f"))
        for ct in range(2):
            nc.sync.dma_start(out=wo_sb[ct][:], in_=wo[ct * 128:(ct + 1) * 128, :])

        loop_stack = ExitStack()

        def loop_gate(phase):
            # enter the device loop right before the first looped phase;
            # close it right before the first non-looped phase after it
            if loop_n > 1 and phase == min(loop_phases):
                loop_stack.enter_context(tc.For_i(0, loop_n))
            if loop_n > 1 and phase == max(loop_phases) + 1:
                loop_stack.close()

        loop_gate(1)
        # ============ phase 1: projections ============
        with tc.tile_pool(name="xin", bufs=10) as xin, \
             tc.tile_pool(name="wsb", bufs=3) as wsb, \
             tc.tile_pool(name="pp1", bufs=2, space="PSUM") as pp1:

            def load_w(src):
                w_t = wsb.tile([128, NPT, DPC], F16, tag="w")
                nc.sync.dma_start(out=w_t[:], in_=src.rearrange("(t p) n -> p t n", p=128))
                return w_t

            def load_x(src):
                tiles = []
                for t8 in range(NPT):
                    x_t = xin.tile([128, S], F16, tag="xin")
                    nc.sync.dma_start(out=x_t[:], in_=src[t8 * 128:(t8 + 1) * 128, :])
                    tiles.append(x_t)
                return tiles

            def proj_T(x_tiles, w_t, out_tiles):
                # out^T[n, q] = W^T @ X^T ; W stationary
                for nt in range(2):
                    for qc in range(NQC):
                        ps = pp1.tile([128, QCW], F32, tag="pp")
                        for t8 in range(NPT):
                            nc.tensor.matmul(
                                ps[:],
                                lhsT=w_t[:, t8, nt * 128:(nt + 1) * 128],
                                rhs=x_tiles[t8][:, qc * QCW:(qc + 1) * QCW],
                                start=(t8 == 0), stop=(t8 == NPT - 1))
                        nc.vector.tensor_copy(
                            out_tiles[nt][:, qc * QCW:(qc + 1) * QCW], ps[:])

            wq_t = load_w(wq)
            q_tiles = load_x(qT)
            proj_T(q_tiles, wq_t, qTp)

            wk_t = load_w(wk)
            k_tiles = load_x(kT)
            proj_T(k_tiles, wk_t, kTp)

            wv_t = load_w(wv)
            v_tiles = load_x(vT)
            # V[m, n] = X @ Wv ; X^T stationary
            for mt in range(NKT):
                ps = pp1.tile([128, DPC], F32, tag="pp")
                for t8 in range(NPT):
                    nc.tensor.matmul(
                        ps[:],
                        lhsT=v_tiles[t8][:, mt * 128:(mt + 1) * 128],
                        rhs=wv_t[:, t8, :],
                        start=(t8 == 0), stop=(t8 == NPT - 1))
                nc.vector.tensor_copy(
                    vp[:, mt, :, 0:HD],
                    ps[:].rearrange("p (h d) -> p h d", h=NH))

        loop_gate(2)
        # ============ phase 2: attention ============
        with tc.tile_pool(name="row", bufs=3) as rowp, \
             tc.tile_pool(name="rbs", bufs=2) as rbp, \
             tc.tile_pool(name="ps2", bufs=2, space="PSUM") as scp, \
             tc.tile_pool(name="xpp", bufs=4, space="PSUM") as xpp:

            for h in range(NH):
                t, r0 = h // 2, 64 * (h % 2)
                xp_tiles = {qc: xpp.tile([65, QCW], F32, tag="xp", name=f"xp{qc}") for qc in range(NQC)}
                for kt in range(NKT):
                    aqcs = [qc for qc in range(NQC) if act[kt, qc]]
                    if not aqcs:
                        continue
                    row = rowp.tile([128, S], F16, tag="row")
                    for qc in aqcs:
                        qs = slice(qc * QCW, (qc + 1) * QCW)
                        ps = scp.tile([128, QCW], F32, tag="sc")
                        nc.tensor.matmul(
                            ps[:],
                            lhsT=kTp[t][r0:r0 + 64, kt * KTW:(kt + 1) * KTW],
                            rhs=qTp[t][r0:r0 + 64, qs],
                            start=True, stop=True)
                        nc.scalar.activation(
                            out=row[:, qs], in_=ps[:], func=AFT.Exp,
                            bias=ebias_sb[:], scale=SCALE)
                        if par[kt, qc]:
                            nc.vector.tensor_mul(
                                row[:, qs], row[:, qs],
                                mpat_sb[:, pidx[(kt, qc)], :])
                        nc.tensor.matmul(
                            xp_tiles[qc][0:65, :],
                            lhsT=vp[:, kt, h, 0:HD + 1],
                            rhs=row[:, qs],
                            start=(kt == first_kt[qc]), stop=(kt == last_kt[qc]))
                    # store contiguous runs of active chunks
                    run0 = aqcs[0]
                    prev = aqcs[0]
                    runs = []
                    for qc in aqcs[1:]:
                        if qc == prev + 1:
                            prev = qc
                        else:
                            runs.append((run0, prev))
                            run0 = prev = qc
                    runs.append((run0, prev))
                    for (a, z) in runs:
                        nc.sync.dma_start(
                            out=probs[h, kt * KTW:(kt + 1) * KTW, a * QCW:(z + 1) * QCW],
                            in_=row[:, a * QCW:(z + 1) * QCW])
                for qc in range(NQC):
                    qs = slice(qc * QCW, (qc + 1) * QCW)
                    xp = xp_tiles[qc]
                    nc.vector.reciprocal(r_sb[0:1, h, qs], xp[64:65, :])
                    with nc.allow_low_precision(reason="fp16 r for broadcast"):
                        nc.vector.reciprocal(r16_sb[0:1, h, qs], xp[64:65, :])
                    rb = scp.tile([64, QCW], F32, tag="sc")
                    nc.tensor.matmul(
                        rb[0:64, :],
                        lhsT=ones_sb[0:1, :],
                        rhs=r16_sb[0:1, h, qs],
                        start=True, stop=True)
                    rbs = rbp.tile([64, QCW], F32, tag="rbs")
                    nc.vector.tensor_copy(rbs[:], rb[0:64, :])
                    nc.vector.tensor_tensor(
                        out=xn[t][r0:r0 + 64, qs], in0=xp[0:64, :], in1=rbs[:],
                        op=mybir.AluOpType.mult)
            nc.sync.dma_start(out=rvec[:, :, :], in_=r_sb[:])

        loop_gate(3)
        # ============ phase 3: output projection ============
        with tc.tile_pool(name="osb", bufs=3) as osbp, \
             tc.tile_pool(name="pp3", bufs=4, space="PSUM") as pp3:
            for mt in range(NKT):
                osb = osbp.tile([128, D], F32, tag="osb")
                for dmc in range(2):
                    ps = pp3.tile([128, 512], F32, tag="op")
                    for ct in range(2):
                        nc.tensor.matmul(
                            ps[:],
                            lhsT=xn[ct][:, mt * 128:(mt + 1) * 128],
                            rhs=wo_sb[ct][:, dmc * 512:(dmc + 1) * 512],
                            start=(ct == 0), stop=(ct == 1))
                    nc.scalar.copy(osb[:, dmc * 512:(dmc + 1) * 512], ps[:])
                nc.sync.dma_start(out=outp[mt * 128:(mt + 1) * 128, :], in_=osb[:])

        loop_gate(4)
        loop_stack.close()

    nc.compile()
    return nc


_CACHE = {}


def _get_program(act, par, plist, n_pat, loop_n=1, loop_phases=(1, 2, 3)):
    key = (act.tobytes(), par.tobytes(), n_pat, loop_n, loop_phases)
    if key not in _CACHE:
        _CACHE[key] = _build(act, par, plist, n_pat, loop_n, loop_phases)
    return _CACHE[key]


def kernel(query, key, value, mask, Wq, Wk, Wv, Wo, bo, _want_trace=False,
           _loop_n=1, _loop_phases=(1, 2, 3)):
    query = np.asarray(query, np.float32)
    key = np.asarray(key, np.float32)
    value = np.asarray(value, np.float32)
    mask = np.asarray(mask)
    Wq = np.asarray(Wq, np.float32)
    Wk = np.asarray(Wk, np.float32)
    Wv = np.asarray(Wv, np.float32)
    Wo = np.asarray(Wo, np.float32)
    bo = np.asarray(bo, np.float32)

    act, par, plist = _block_structure(mask != 0)
    n_pat = max(len(plist), 1)
    nc = _get_program(act, par, plist, n_pat, _loop_n, _loop_phases)

    in_maps = []
    for b in range(B):
        qTb = query[b].T.astype(np.float16)
        kTb = key[b].T.astype(np.float16)
        vTb = value[b].T.astype(np.float16)
        if plist:
            mp = np.stack([
                mask[b, qc * QCW:(qc + 1) * QCW, kt * KTW:(kt + 1) * KTW].T
                for (kt, qc) in plist]).astype(np.float16)
        else:
            mp = np.ones((1, KTW, QCW), np.float16)
        for g in range(G):
            in_maps.append({
                "qT": qTb, "kT": kTb, "vT": vTb,
                "wq": Wq[:, g * DPC:(g + 1) * DPC].astype(np.float16),
                "wk": Wk[:, g * DPC:(g + 1) * DPC].astype(np.float16),
                "wv": Wv[:, g * DPC:(g + 1) * DPC].astype(np.float16),
                "wo": Wo[g * DPC:(g + 1) * DPC, :].astype(np.float16),
                "mpat": mp,
            })

    kwargs = {}
    if _want_trace:
        kwargs = dict(trace=True, trace_cores=[0])
    import time as _time
    _t0 = _time.time()
    res = run_bass_kernel_spmd(nc, in_maps, core_ids=list(range(8)), **kwargs)
    kernel._last_exec_wall = _time.time() - _t0

    out = np.zeros((B, S, D), np.float32)
    attn = np.zeros((B, H, S, S), np.float32)
    for c in range(8):
        b, g = divmod(c, G)
        rr = res.results[c]
        out[b] += rr["outp"]
        rv = rr["rvec"].reshape(NH, S)
        pr = rr["probs"]
        for lh in range(NH):
            h = g * NH + lh
            a = pr[lh].astype(np.float32)
            a *= rv[lh][None, :]
            attn[b, h] = a.T
    out += bo[None, None, :]
    if _want_trace:
        kernel._last_results = res
    return out, attn


# revision 19
# speedup vs baseline: 25.0675x; 1.7796x over previous
"""Trainium2 Bass kernel for causal multi-head attention (B=2,S=2048,D=1024,H=16).

Sharding: batch*head-group across 8 cores. Core c = (b, g) = (c//4, c%4) computes
heads [4g, 4g+4) of batch b.

Device computes, per core (fp16 matmuls, fp32 PSUM):
  QT/KT = (X @ W)^T via W-stationary matmuls    -> [256, 2048] fp16 in SBUF
  V     = X @ Wv via XT-stationary matmuls      -> [2048, 4, 68] fp16 (+ones col)
  scoresT[k, q] = K_h @ Q_h^T (K=64 contraction), exp on ScalarE (scale=1/8,
  bias=-4), partial causal blocks masked by multiplying 0/1 patterns,
  PV matmul with ones-column -> x'^T[65, q] where row 64 = softmax denominator l,
  r = 1/l, broadcast via ones-outer-product matmul, x^T normalized,
  out_partial = x^T.T @ Wo_rows.
Host: gathers fp16 probs (unnormalized, [k,q] layout), multiplies by r,
transposes to [q,k]; sums the 4 partial outputs per batch and adds bias.
"""

import numpy as np
from contextlib import ExitStack

import concourse.bacc as bacc
import concourse.tile as tile
from concourse import mybir
from concourse.bass_utils import run_bass_kernel_spmd

B, S, D, H, HD = 2, 2048, 1024, 16, 64
NH = 4                # heads per core
G = H // NH           # head groups (cores per batch)
DPC = NH * HD         # 256 output dims per core
NKT, KTW = 16, 128    # key tiles
NQC, QCW = 4, 512     # query chunks
NPT = D // 128        # projection contraction tiles (8)
VPW = 68              # per-head V stride (64 data + 1 ones + 3 pad, 4B aligned)
F16 = mybir.dt.float16
F32 = mybir.dt.float32
F32R = mybir.dt.float32r
EXP_BIAS = -4.0
SCALE = 1.0 / 8.0     # 1/sqrt(HD)

AFT = mybir.ActivationFunctionType


def _block_structure(mask):
    """Classify (kt, qc) blocks from the actual mask values.

    Returns (act, par, plist):
      act[kt][qc]  - block has any attendable entry in any batch
      par[kt][qc]  - block needs masking (not all-ones in every batch)
      plist        - ordered list of (kt, qc) partial blocks
    """
    m = mask.reshape(B, NQC, QCW, NKT, KTW)
    any_b = m.any(axis=(2, 4))   # [B, NQC, NKT]
    all_b = m.all(axis=(2, 4))
    act = any_b.any(axis=0).T    # [NKT, NQC]
    full = all_b.all(axis=0).T
    par = act & ~full
    plist = [(kt, qc) for kt in range(NKT) for qc in range(NQC) if par[kt, qc]]
    return act, par, plist


def _build(act, par, plist, n_pat, loop_n=1, loop_phases=(1, 2, 3)):
    pidx = {kq: i for i, kq in enumerate(plist)}
    first_kt = {}
    last_kt = {}
    for qc in range(NQC):
        kts = [kt for kt in range(NKT) if act[kt, qc]]
        assert kts, f"query chunk {qc} has no attendable keys"
        first_kt[qc], last_kt[qc] = kts[0], kts[-1]

    nc = bacc.Bacc("TRN2", target_bir_lowering=False, debug=False, num_devices=8)
    qT = nc.dram_tensor("qT", [D, S], F16, kind="ExternalInput").ap()
    kT = nc.dram_tensor("kT", [D, S], F16, kind="ExternalInput").ap()
    vT = nc.dram_tensor("vT", [D, S], F16, kind="ExternalInput").ap()
    wq = nc.dram_tensor("wq", [D, DPC], F16, kind="ExternalInput").ap()
    wk = nc.dram_tensor("wk", [D, DPC], F16, kind="ExternalInput").ap()
    wv = nc.dram_tensor("wv", [D, DPC], F16, kind="ExternalInput").ap()
    wo = nc.dram_tensor("wo", [DPC, D], F16, kind="ExternalInput").ap()
    mpat = nc.dram_tensor("mpat", [n_pat, KTW, QCW], F16, kind="ExternalInput").ap()
    probs = nc.dram_tensor("probs", [NH, S, S], F16, kind="ExternalOutput").ap()
    outp = nc.dram_tensor("outp", [S, D], F16, kind="ExternalOutput").ap()
    rvec = nc.dram_tensor("rvec", [1, NH, S], F32, kind="ExternalOutput").ap()

    with tile.TileContext(nc) as tc, ExitStack() as ctx:
        singles = ctx.enter_context(tc.tile_pool(name="singles", bufs=1))

        # --- persistent tiles ---
        qTp = [singles.tile([128, S], F16, tag=f"qTp{t}", name=f"qTp{t}") for t in range(2)]
        kTp = [singles.tile([128, S], F16, tag=f"kTp{t}", name=f"kTp{t}") for t in range(2)]
        vp = singles.tile([128, NKT, NH, VPW], F16, tag="vp")
        xn = [singles.tile([128, S], F16, tag=f"xn{t}", name=f"xn{t}") for t in range(2)]
        mpat_sb = singles.tile([128, n_pat, QCW], F16, tag="mpat")
        r_sb = singles.tile([1, NH, S], F32, tag="r")
        ones_sb = singles.tile([1, HD], F16, tag="ones")
        r16_sb = singles.tile([1, NH, S], F16, tag="r16")
        ebias_sb = singles.tile([128, 1], F32, tag="ebias")
        wo_sb = [singles.tile([128, D], F16, tag=f"wo{t}", name=f"wo{t}") for t in range(2)]

        nc.vector.memset(ones_sb[:], 1.0)
        nc.vector.memset(ebias_sb[:], EXP_BIAS)
        nc.vector.memset(vp[:, :, :, HD:HD + 1], 1.0)
        nc.sync.dma_start(out=mpat_sb[:], in_=mpat.rearrange("n p f -> p n f"))
        for ct in range(2):
            nc.sync.dma_start(out=wo_sb[ct][:], in_=wo[ct * 128:(ct + 1) * 128, :])

        loop_stack = ExitStack()

        def loop_gate(phase):
            # enter the device loop right before the first looped phase;
            # close it right before the first non-looped phase after it
            if loop_n > 1 and phase == min(loop_phases):
                loop_stack.enter_context(tc.For_i(0, loop_n))
            if loop_n > 1 and phase == max(loop_phases) + 1:
                loop_stack.close()

        pp1 = ctx.enter_context(tc.tile_pool(name="pp1", bufs=2, space="PSUM"))
        scp = ctx.enter_context(tc.tile_pool(name="ps2", bufs=2, space="PSUM"))
        xpp = ctx.enter_context(tc.tile_pool(name="xpp", bufs=4, space="PSUM"))

        loop_gate(1)
        # ============ phase 1: projections ============
        with tc.tile_pool(name="xin", bufs=10) as xin, \
             tc.tile_pool(name="wsb", bufs=3) as wsb:

            def load_w(src):
                w_t = wsb.tile([128, NPT, DPC], F16, tag="w")
                nc.sync.dma_start(out=w_t[:], in_=src.rearrange("(t p) n -> p t n", p=128))
                return w_t

            def load_x(src):
                tiles = []
                for t8 in range(NPT):
                    x_t = xin.tile([128, S], F16, tag="xin")
                    nc.sync.dma_start(out=x_t[:], in_=src[t8 * 128:(t8 + 1) * 128, :])
                    tiles.append(x_t)
                return tiles

            def proj_T(x_tiles, w_t, out_tiles):
                # out^T[n, q] = W^T @ X^T ; W stationary
                for nt in range(2):
                    for qc in range(NQC):
                        ps = pp1.tile([128, QCW], F32, tag="pp")
                        for t8 in range(NPT):
                            nc.tensor.matmul(
                                ps[:],
                                lhsT=w_t[:, t8, nt * 128:(nt + 1) * 128],
                                rhs=x_tiles[t8][:, qc * QCW:(qc + 1) * QCW],
                                start=(t8 == 0), stop=(t8 == NPT - 1))
                        nc.vector.tensor_copy(
                            out_tiles[nt][:, qc * QCW:(qc + 1) * QCW], ps[:])

            wq_t = load_w(wq)
            q_tiles = load_x(qT)
            proj_T(q_tiles, wq_t, qTp)

            wk_t = load_w(wk)
            k_tiles = load_x(kT)
            proj_T(k_tiles, wk_t, kTp)

            wv_t = load_w(wv)
            v_tiles = load_x(vT)
            # V[m, n] = X @ Wv ; X^T stationary
            for mt in range(NKT):
                ps = pp1.tile([128, DPC], F32, tag="pp")
                for t8 in range(NPT):
                    nc.tensor.matmul(
                        ps[:],
                        lhsT=v_tiles[t8][:, mt * 128:(mt + 1) * 128],
                        rhs=wv_t[:, t8, :],
                        start=(t8 == 0), stop=(t8 == NPT - 1))
                nc.vector.tensor_copy(
                    vp[:, mt, :, 0:HD],
                    ps[:].rearrange("p (h d) -> p h d", h=NH))

        loop_gate(2)
        # ============ phase 2: attention ============
        with tc.tile_pool(name="row", bufs=3) as rowp, \
             tc.tile_pool(name="xsb", bufs=8) as xsbp:

            for h in range(NH):
                t, r0 = h // 2, 64 * (h % 2)
                xp_tiles = {qc: xpp.tile([65, QCW], F32, tag="xp", name=f"xp{qc}")
                            for qc in range(NQC)}
                rows = {}
                kts = [kt for kt in range(NKT)
                       if any(act[kt, qc] for qc in range(NQC))]

                def emit_scores(kt):
                    aqcs = [qc for qc in range(NQC) if act[kt, qc]]
                    row = rowp.tile([128, S], F16, tag="row", name="row")
                    rows[kt] = row
                    for qc in aqcs:
                        qs = slice(qc * QCW, (qc + 1) * QCW)
                        ps = scp.tile([128, QCW], F32, tag="sc", name="sc")
                        nc.tensor.matmul(
                            ps[:],
                            lhsT=kTp[t][r0:r0 + 64, kt * KTW:(kt + 1) * KTW],
                            rhs=qTp[t][r0:r0 + 64, qs],
                            start=True, stop=True)
                        nc.scalar.activation(
                            out=row[:, qs], in_=ps[:], func=AFT.Exp,
                            bias=ebias_sb[:], scale=SCALE)
                        if par[kt, qc]:
                            nc.vector.tensor_mul(
                                row[:, qs], row[:, qs],
                                mpat_sb[:, pidx[(kt, qc)], :])
                    # store contiguous runs of active chunks
                    runs = []
                    run0 = prev_qc = aqcs[0]
                    for qc in aqcs[1:]:
                        if qc == prev_qc + 1:
                            prev_qc = qc
                        else:
                            runs.append((run0, prev_qc))
                            run0 = prev_qc = qc
                    runs.append((run0, prev_qc))
                    for (a, z) in runs:
                        nc.sync.dma_start(
                            out=probs[h, kt * KTW:(kt + 1) * KTW,
                                      a * QCW:(z + 1) * QCW],
                            in_=rows[kt][:, a * QCW:(z + 1) * QCW])

                def emit_pv(kt):
                    for qc in range(NQC):
                        if not act[kt, qc]:
                            continue
                        qs = slice(qc * QCW, (qc + 1) * QCW)
                        nc.tensor.matmul(
                            xp_tiles[qc][0:65, :],
                            lhsT=vp[:, kt, h, 0:HD + 1],
                            rhs=rows[kt][:, qs],
                            start=(kt == first_kt[qc]), stop=(kt == last_kt[qc]))

                prev = None
                for kt in kts:
                    emit_scores(kt)
                    if prev is not None:
                        emit_pv(prev)
                    prev = kt
                emit_pv(prev)

                for qc in range(NQC):
                    qs = slice(qc * QCW, (qc + 1) * QCW)
                    xp = xp_tiles[qc]
                    xsb = xsbp.tile([65, QCW], F32, tag="xsb", name="xsb")
                    nc.vector.tensor_copy(xsb[:], xp[0:65, :])
                    nc.vector.reciprocal(r_sb[0:1, h, qs], xsb[64:65, :])
                    with nc.allow_low_precision(reason="fp16 r for broadcast"):
                        nc.vector.reciprocal(r16_sb[0:1, h, qs], xsb[64:65, :])
                    rb = scp.tile([64, QCW], F32, tag="sc", name="rb")
                    nc.tensor.matmul(
                        rb[0:64, :],
                        lhsT=ones_sb[0:1, :],
                        rhs=r16_sb[0:1, h, qs],
                        start=True, stop=True)
                    nc.vector.tensor_tensor(
                        out=xn[t][r0:r0 + 64, qs], in0=xsb[0:64, :],
                        in1=rb[0:64, :], op=mybir.AluOpType.mult)
            nc.sync.dma_start(out=rvec[:, :, :], in_=r_sb[:])

        loop_gate(3)
        # ============ phase 3: output projection ============
        with tc.tile_pool(name="osb", bufs=3) as osbp:
            for mt in range(NKT):
                osb = osbp.tile([128, D], F16, tag="osb", name="osb")
                for dmc in range(2):
                    ps = pp1.tile([128, 512], F32, tag="pp", name="ops")
                    for ct in range(2):
                        nc.tensor.matmul(
                            ps[:],
                            lhsT=xn[ct][:, mt * 128:(mt + 1) * 128],
                            rhs=wo_sb[ct][:, dmc * 512:(dmc + 1) * 512],
                            start=(ct == 0), stop=(ct == 1))
                    nc.scalar.copy(osb[:, dmc * 512:(dmc + 1) * 512], ps[:])
                nc.sync.dma_start(out=outp[mt * 128:(mt + 1) * 128, :], in_=osb[:])

        loop_gate(4)
        loop_stack.close()

    nc.compile()
    return nc


_CACHE = {}


def _get_program(act, par, plist, n_pat, loop_n=1, loop_phases=(1, 2, 3)):
    key = (act.tobytes(), par.tobytes(), n_pat, loop_n, loop_phases)
    if key not in _CACHE:
        _CACHE[key] = _build(act, par, plist, n_pat, loop_n, loop_phases)
    return _CACHE[key]


def kernel(query, key, value, mask, Wq, Wk, Wv, Wo, bo, _want_trace=False,
           _loop_n=1, _loop_phases=(1, 2, 3)):
    query = np.asarray(query, np.float32)
    key = np.asarray(key, np.float32)
    value = np.asarray(value, np.float32)
    mask = np.asarray(mask)
    Wq = np.asarray(Wq, np.float32)
    Wk = np.asarray(Wk, np.float32)
    Wv = np.asarray(Wv, np.float32)
    Wo = np.asarray(Wo, np.float32)
    bo = np.asarray(bo, np.float32)

    act, par, plist = _block_structure(mask != 0)
    n_pat = max(len(plist), 1)
    nc = _get_program(act, par, plist, n_pat, _loop_n, _loop_phases)

    in_maps = []
    for b in range(B):
        qTb = query[b].T.astype(np.float16)
        kTb = key[b].T.astype(np.float16)
        vTb = value[b].T.astype(np.float16)
        if plist:
            mp = np.stack([
                mask[b, qc * QCW:(qc + 1) * QCW, kt * KTW:(kt + 1) * KTW].T
                for (kt, qc) in plist]).astype(np.float16)
        else:
            mp = np.ones((1, KTW, QCW), np.float16)
        for g in range(G):
            in_maps.append({
                "qT": qTb, "kT": kTb, "vT": vTb,
                "wq": Wq[:, g * DPC:(g + 1) * DPC].astype(np.float16),
                "wk": Wk[:, g * DPC:(g + 1) * DPC].astype(np.float16),
                "wv": Wv[:, g * DPC:(g + 1) * DPC].astype(np.float16),
                "wo": Wo[g * DPC:(g + 1) * DPC, :].astype(np.float16),
                "mpat": mp,
            })

    kwargs = {}
    if _want_trace:
        kwargs = dict(trace=True, trace_cores=[0])
    import time as _time
    _t0 = _time.time()
    res = run_bass_kernel_spmd(nc, in_maps, core_ids=list(range(8)), **kwargs)
    kernel._last_exec_wall = _time.time() - _t0

    out = np.zeros((B, S, D), np.float32)
    attn = np.zeros((B, H, S, S), np.float32)
    for c in range(8):
        b, g = divmod(c, G)
        rr = res.results[c]
        out[b] += rr["outp"].astype(np.float32)
        rv = rr["rvec"].reshape(NH, S)
        pr = rr["probs"]
        for lh in range(NH):
            h = g * NH + lh
            a = pr[lh].astype(np.float32)
            a *= rv[lh][None, :]
            attn[b, h] = a.T
    out += bo[None, None, :]
    if _want_trace:
        kernel._last_results = res
    return out, attn
